# revision 10
# baseline (speedup 1.0000x reference)
"""Trainium2 Bass kernel for ColorImageLoss (gaussian-blur + bilinear grid
sample + MSE), data-parallel over batch across 8 NeuronCores.

Key idea: the loss only reads the blurred image at 64 sample points per
image.  Each bilinear sample needs a 2x2 patch of blurred pixels; the 7-tap
separable blur support of those pixels is an 8x8 patch of the *original*
image (reflect padding folds into per-sample 8-tap row/col weight vectors;
reflected tap indices provably stay inside the clamped 8-wide window
[clamp(x0-3,0,W-8), +8)).

Gather strategy: the host pre-builds a *banded* replica of each image —
for every window row start ys in [0,505) an 8-row band stored as
[x][c][r] — so a full 8x8x3 patch is ONE contiguous 768B run at element
offset ((img*505+ys)*512+xs)*24.  The kernel then needs only SLOTS(=2)
indirect-DMA calls (one 768B descriptor per partition each) instead of
48 calls of 32B descriptors; SWDGE per-call overhead dominated the old
58us kernel.  Weights are applied separably (row-combine then
col-combine) to keep every access pattern <=3 free dims.
"""

import os
import sys

import numpy as np

for _p in ("/opt/trn_rl_repo", "/root/.axon_site/_ro/trn_rl_repo"):
    if os.path.isdir(_p) and _p not in sys.path:
        sys.path.insert(0, _p)

import concourse.bass as bass
import concourse.mybir as mybir
import concourse.tile as tile
from concourse.bass_utils import run_bass_kernel_spmd

# Problem geometry (hardcoded per contract)
B, L, NCH, H, W = 32, 64, 3, 512, 512
NCORES = 8
BPC = B // NCORES            # images per core
NS = BPC * L                 # samples per core (256)
P = 128                      # SBUF partitions
SLOTS = NS // P              # 2 sample slots per partition
KS = 7                       # blur taps
BANDS = H - 8 + 1            # 505 window row starts
BAND_ROW = W * NCH * 8       # 12288 elems per (img, ys)
IMG_BASE = BANDS * BAND_ROW  # 6205440 elems per image
IMG_ELEMS = BPC * IMG_BASE   # banded elems per core
PATCH = 8 * NCH * 8          # 192 elems per gathered patch

f32 = mybir.dt.float32
i32 = mybir.dt.int32
Alu = mybir.AluOpType
Ax = mybir.AxisListType

# meta tensor per-partition layout (f32 columns)
O_POS = 0             # [SLOTS, 2] (x, y)                  -> 4
O_COL = 4             # [SLOTS, 3] color                   -> 6
O_JMW = 10            # [2which, 7] j-3+which              -> 14
O_KK = 24             # [7] blur kernel                    -> 7
O_IOTA8 = 31          # [8] 0..7                           -> 8
O_SCALE = 39          # (24, 12288) x/y index scales       -> 2
O_BASE = 41           # [SLOTS] i32 image base (bit-cast)  -> 2
META_W = 44


def _gauss_kernel_np():
    x = (np.arange(KS, dtype=np.float32) - (KS - 1) / 2).astype(np.float32)
    k = np.exp(-0.5 * (x / np.float32(1.0)) ** 2).astype(np.float32)
    return (k / k.sum()).astype(np.float32)


def _fap(t, dims, extra_offset=0):
    """AP over tile `t` keeping its partition dim, replacing free dims.

    dims: list of [step, count] in elements; step 0 broadcasts.
    """
    base = t[:] if hasattr(t, "tile") else t
    return bass.AP(
        base.tensor, base.offset + extra_offset,
        [list(base.ap[0])] + [list(d) for d in dims],
    )


def split_multi_waits(nc):
    """Walrus encodes at most ONE sync wait per TPB instruction.  Hoist
    extra waits onto same-engine NoOps inserted directly before the
    instruction (the sequencer executes waits in queue order, so semantics
    are identical)."""
    n_split = 0
    for f in nc.m.functions:
        for blk in f.blocks:
            insts = blk.instructions
            i = 0
            while i < len(insts):
                inst = insts[i]
                si = inst.sync_info
                if si is not None and si.on_wait is not None and len(si.on_wait) > 1:
                    waits = list(si.on_wait)
                    for w in waits[:-1]:
                        nop = mybir.InstNoOp(
                            name=f"{inst.name}-wsplit{n_split}",
                            engine=inst.engine,
                            ins=[],
                            outs=[],
                            sync_info=mybir.SyncInfo(on_wait=[w], on_update=[]),
                        )
                        nc.register_instruction(nop, overwrite=True)
                        insts.insert(i, nop)
                        i += 1
                        n_split += 1
                    inst.sync_info = mybir.SyncInfo(
                        on_wait=[waits[-1]], on_update=list(si.on_update or []))
                i += 1
    return n_split


def build_bass(debug_taps=False, repeat=1):
    assert not (debug_taps and repeat > 1)
    nc = bass.Bass("TRN2")

    img = nc.dram_tensor("img", [IMG_ELEMS, 1], f32, kind="ExternalInput")
    meta = nc.dram_tensor("meta", [P, META_W], f32, kind="ExternalInput")
    partial = nc.dram_tensor("partial", [P, 1], f32, kind="ExternalOutput")

    taps = []

    def tap(name, t, width, dt=f32):
        if not debug_taps:
            return
        d = nc.dram_tensor(f"tap_{name}", [P, width], dt, kind="ExternalOutput")
        taps.append((name, t, d, width))

    with tile.TileContext(nc) as tc:
        with tc.tile_pool(name="main", bufs=1) as pool:
            # tiles allocated ONCE; `repeat` bodies reuse them so reps
            # serialize through buffer dependencies (honest latency bench)
            m = pool.tile([P, META_W], f32)
            xy = pool.tile([P, SLOTS, 2], f32)
            s2 = pool.tile([P, SLOTS, 2], f32)
            offf = pool.tile([P, SLOTS, 2], f32)
            off = pool.tile([P, SLOTS], f32)
            offi = pool.tile([P, SLOTS], i32)
            idx = pool.tile([P, SLOTS], i32)
            patches = pool.tile([P, SLOTS, PATCH], f32)
            wxy = pool.tile([P, SLOTS, 2], f32)
            fxy = pool.tile([P, SLOTS, 2], f32)
            rnd = pool.tile([P, SLOTS, 2], f32)
            gtx = pool.tile([P, SLOTS, 2], f32)
            tt = pool.tile([P, SLOTS, 2, 2 * KS], f32)
            ta = pool.tile([P, SLOTS, 2, 2 * KS], f32)
            tb = pool.tile([P, SLOTS, 2, 2 * KS], f32)
            tr = pool.tile([P, SLOTS, 2, 2 * KS], f32)
            tz = pool.tile([P, SLOTS, 2, 2 * KS], f32)
            eq = pool.tile([P, SLOTS * 4, 8, KS], f32)
            cw = pool.tile([P, SLOTS, 2, 2, KS], f32)
            eqk = pool.tile([P, SLOTS * 4, 8, KS], f32)
            vh = pool.tile([P, SLOTS, 2, 2, 8], f32)
            hwv = pool.tile([P, SLOTS, 2, 8], f32)
            t1 = pool.tile([P, SLOTS, 24, 8], f32)
            t1r = pool.tile([P, SLOTS, 24], f32)
            t2 = pool.tile([P, SLOTS, NCH, 8], f32)
            tgt = pool.tile([P, SLOTS, NCH], f32)
            diff = pool.tile([P, SLOTS, NCH], f32)
            sqj = pool.tile([P, SLOTS, NCH], f32)
            part = pool.tile([P, 1], f32)

            for _rep in range(repeat):
                nc.sync.dma_start(out=m[:], in_=meta[:])

                # ---- index path (gather critical path) ----
                # x = clip(pos*512 - 0.5, 0, 511)  (continuous in pos, so
                # f32 reassociation vs the reference op order is harmless)
                pos_ap = _fap(m, [[2, SLOTS], [1, 2]], O_POS)
                nc.vector.tensor_scalar(xy[:], pos_ap, float(W), -0.5,
                                        Alu.mult, Alu.add)
                nc.vector.tensor_scalar(xy[:], xy[:], 0.0, float(W - 1),
                                        Alu.max, Alu.min)
                tap('xy', xy, 4)
                # floor via round-to-nearest at 2^23 (f32 grid is 1.0 there),
                # minus the rounded-up-past-x case
                nc.vector.tensor_scalar(rnd[:], xy[:], 8388608.0, None, Alu.add)
                nc.vector.tensor_scalar(rnd[:], rnd[:], -8388608.0, None,
                                        Alu.add)
                nc.vector.tensor_tensor(gtx[:], rnd[:], xy[:], op=Alu.is_gt)
                nc.vector.tensor_tensor(fxy[:], rnd[:], gtx[:],
                                        op=Alu.subtract)
                # window start s2 = clamp(floor(x)-3, 0, 504), integral f32
                nc.vector.tensor_scalar(s2[:], fxy[:], -3.0, 0.0,
                                        Alu.add, Alu.max)
                nc.vector.tensor_scalar(s2[:], s2[:], float(W - 8), None,
                                        Alu.min)
                # element offset = sx*24 + sy*12288 (exact in f32: <2^24)
                scale_ap = _fap(m, [[0, SLOTS], [1, 2]], O_SCALE)
                nc.vector.tensor_tensor(offf[:], s2[:], scale_ap, op=Alu.mult)
                nc.vector.tensor_tensor(
                    off[:], _fap(offf, [[2, SLOTS], [1, 1]]),
                    _fap(offf, [[2, SLOTS], [1, 1]], 1), op=Alu.add)
                # i32 trunc of s2 is exact (integral); add per-slot img base
                nc.vector.tensor_copy(offi[:], off[:])
                base_ap = _fap(m, [[1, SLOTS]], O_BASE).bitcast(i32)
                nc.vector.tensor_tensor(idx[:], offi[:], base_ap, op=Alu.add)
                tap('idx', idx, SLOTS, i32)

                # ---- gather: one 768B descriptor per partition per slot ----
                for slot in range(SLOTS):
                    nc.gpsimd.indirect_dma_start(
                        out=_fap(patches, [[1, PATCH]], PATCH * slot),
                        out_offset=None,
                        in_=img[:],
                        in_offset=bass.IndirectOffsetOnAxis(
                            ap=_fap(idx, [[1, 1]], slot), axis=0),
                    )

                # ---- weight path (overlaps gather DMA) ----
                nc.vector.tensor_tensor(wxy[:], xy[:], fxy[:], op=Alu.subtract)
                tap('fxy', fxy, 4)
                # raw taps t[s, ax, w, j] = floor + (j-3+w)
                jmw_ap = _fap(m, [[0, SLOTS], [0, 2], [1, 2 * KS]], O_JMW)
                fxy_b = _fap(fxy, [[2, SLOTS], [1, 2], [0, 2 * KS]])
                nc.vector.tensor_tensor(
                    _fap(tt, [[4 * KS, SLOTS], [2 * KS, 2], [1, 2 * KS]]),
                    fxy_b, jmw_ap, op=Alu.add)
                # reflect: r = min(|t|, 2*(W-1) - t);  |t| = max(t, -t)
                nc.vector.tensor_scalar(ta[:], tt[:], -1.0, None, Alu.mult)
                nc.vector.tensor_tensor(ta[:], ta[:], tt[:], op=Alu.max)
                nc.vector.tensor_scalar(tb[:], tt[:], -1.0, float(2 * (W - 1)),
                                        Alu.mult, Alu.add)
                nc.vector.tensor_tensor(tr[:], ta[:], tb[:], op=Alu.min)
                # window-relative taps z = r - s(ax)  (in [0,8) iff covered)
                s2_b = _fap(s2, [[2, SLOTS], [1, 2], [0, 2 * KS]])
                nc.vector.tensor_tensor(
                    _fap(tz, [[4 * KS, SLOTS], [2 * KS, 2], [1, 2 * KS]]),
                    _fap(tr, [[4 * KS, SLOTS], [2 * KS, 2], [1, 2 * KS]]),
                    s2_b, op=Alu.subtract)
                tap('tz', tz, 4 * KS * SLOTS)
                # eq[g, u, j] = (z[g, j] == u),  g = (s, ax, w)
                G = SLOTS * 4
                z_b = _fap(tz, [[KS, G], [0, 8], [1, KS]])
                iota_b = _fap(m, [[0, G], [1, 8], [0, KS]], O_IOTA8)
                nc.vector.tensor_tensor(eq[:], z_b, iota_b, op=Alu.is_equal)
                # cw[s, ax, w, j] = kk[j] * (w ? wxy : 1-wxy)
                kk_b = _fap(m, [[0, SLOTS], [0, 2], [1, KS]], O_KK)
                wxy_b = _fap(wxy, [[2, SLOTS], [1, 2], [0, KS]])
                cw1 = _fap(cw, [[4 * KS, SLOTS], [2 * KS, 2], [1, KS]], KS)
                nc.vector.tensor_tensor(cw1, kk_b, wxy_b, op=Alu.mult)
                nc.vector.tensor_tensor(
                    _fap(cw, [[4 * KS, SLOTS], [2 * KS, 2], [1, KS]]),
                    kk_b, cw1, op=Alu.subtract)
                # vh[g, u] = sum_j eq * cw ; hwv[s, ax, u] = vh_w0 + vh_w1
                cw_b = _fap(cw, [[KS, G], [0, 8], [1, KS]])
                nc.vector.tensor_tensor(eqk[:], eq[:], cw_b, op=Alu.mult)
                nc.vector.tensor_reduce(
                    out=_fap(vh, [[1, G * 8]]), in_=eqk[:], axis=Ax.X,
                    op=Alu.add)
                nc.vector.tensor_tensor(
                    hwv[:],
                    _fap(vh, [[32, SLOTS], [16, 2], [1, 8]]),
                    _fap(vh, [[32, SLOTS], [16, 2], [1, 8]], 8), op=Alu.add)
                tap('hwv', hwv, SLOTS * 16)

                # ---- apply weights separably; patch layout [x][c][r] ----
                tap('patches', patches, SLOTS * PATCH)
                vw_b = _fap(hwv, [[16, SLOTS], [0, 24], [1, 8]], 8)
                pat_v = _fap(patches, [[PATCH, SLOTS], [8, 24], [1, 8]])
                nc.vector.tensor_tensor(t1[:], pat_v, vw_b, op=Alu.mult)
                nc.vector.tensor_reduce(
                    out=_fap(t1r, [[1, SLOTS * 24]]), in_=t1[:], axis=Ax.X,
                    op=Alu.add)
                hw_b = _fap(hwv, [[16, SLOTS], [0, NCH], [1, 8]])
                t1r_v = _fap(t1r, [[24, SLOTS], [1, NCH], [NCH, 8]])
                nc.vector.tensor_tensor(
                    _fap(t2, [[NCH * 8, SLOTS], [8, NCH], [1, 8]]),
                    t1r_v, hw_b, op=Alu.mult)
                nc.vector.tensor_reduce(
                    out=_fap(tgt, [[1, SLOTS * NCH]]), in_=t2[:], axis=Ax.X,
                    op=Alu.add)
                tap('tgt', tgt, SLOTS * NCH)
                col_ap = _fap(m, [[NCH, SLOTS], [1, NCH]], O_COL)
                nc.vector.tensor_tensor(diff[:], tgt[:], col_ap,
                                        op=Alu.subtract)
                nc.vector.tensor_tensor(sqj[:], diff[:], diff[:], op=Alu.mult)
                nc.vector.tensor_reduce(
                    out=part[:], in_=_fap(sqj, [[1, SLOTS * NCH]]), axis=Ax.X,
                    op=Alu.add)

                nc.sync.dma_start(out=partial[:], in_=part[:])

                for _name, _t, _d, _w in taps:
                    nc.sync.dma_start(out=_d[:], in_=_fap(_t, [[1, _w]]))

    split_multi_waits(nc)
    return nc


def make_meta(pred_shard):
    """Build the per-core [P, META_W] meta tensor from the [BPC, L, 8]
    predictions shard.  Sample i = slot*P + p."""
    flat = np.ascontiguousarray(pred_shard.reshape(NS, 8).astype(np.float32))
    meta = np.zeros((P, META_W), dtype=np.float32)
    pos = flat[:, :2].reshape(SLOTS, P, 2).transpose(1, 0, 2)     # [P,SLOTS,2]
    col = flat[:, 5:8].reshape(SLOTS, P, 3).transpose(1, 0, 2)    # [P,SLOTS,3]
    meta[:, O_POS:O_POS + 4] = pos.reshape(P, 4)
    meta[:, O_COL:O_COL + 6] = col.reshape(P, 6)
    jmw = np.concatenate([np.arange(KS, dtype=np.float32) - 3.0 + w
                          for w in (0.0, 1.0)])                   # [2*7]
    meta[:, O_JMW:O_JMW + 14] = jmw[None, :]
    meta[:, O_KK:O_KK + KS] = _gauss_kernel_np()[None, :]
    meta[:, O_IOTA8:O_IOTA8 + 8] = np.arange(8, dtype=np.float32)[None, :]
    meta[:, O_SCALE] = float(NCH * 8)        # x scale (24)
    meta[:, O_SCALE + 1] = float(BAND_ROW)   # y scale (12288)
    p_idx = np.arange(P)
    base = np.zeros((P, SLOTS), dtype=np.int32)
    for slot in range(SLOTS):
        base[:, slot] = ((slot * P + p_idx) // L).astype(np.int32) * IMG_BASE
    meta[:, O_BASE:O_BASE + SLOTS] = base.view(np.float32)
    return meta


def make_banded(ref_imgs):
    """banded[i, ys, x, c, r] = img[i, c, ys+r, x]  — an 8x8x3 patch at
    (ys, xs) is the contiguous 192-elem run at ((i*505+ys)*512+xs)*24."""
    imgs = np.asarray(ref_imgs, dtype=np.float32)
    Y = np.ascontiguousarray(imgs.transpose(0, 2, 3, 1))  # [B, y, x, c]
    banded = np.empty((B, BANDS, W, NCH, 8), dtype=np.float32)
    for r in range(8):
        banded[:, :, :, :, r] = Y[:, r:r + BANDS]
    return banded


def make_in_maps(predictions, ref_imgs):
    banded = make_banded(ref_imgs)
    in_maps = []
    for k in range(NCORES):
        shard = np.ascontiguousarray(
            banded[k * BPC:(k + 1) * BPC]).reshape(-1, 1)
        meta = make_meta(predictions[k * BPC:(k + 1) * BPC])
        in_maps.append({"img": shard, "meta": meta})
    return in_maps


_NC_CACHE = {}


def get_nc():
    if "nc" not in _NC_CACHE:
        _NC_CACHE["nc"] = build_bass()
    return _NC_CACHE["nc"]


def _reduce_results(res):
    total = np.float64(0.0)
    for r in res.results:
        total += np.float64(r["partial"].sum(dtype=np.float64))
    return np.float32(total / (B * L * NCH))


def kernel(predictions, ref_imgs):
    predictions = np.asarray(predictions)
    ref_imgs = np.asarray(ref_imgs)
    nc = get_nc()
    in_maps = make_in_maps(predictions, ref_imgs)
    res = run_bass_kernel_spmd(nc, in_maps, list(range(NCORES)))
    return _reduce_results(res)


def run_profiled(predictions, ref_imgs):
    """Like kernel(), but traces with neuron-profile; returns (loss, results)."""
    predictions = np.asarray(predictions)
    ref_imgs = np.asarray(ref_imgs)
    nc = get_nc()
    in_maps = make_in_maps(predictions, ref_imgs)
    res = run_bass_kernel_spmd(
        nc, in_maps, list(range(NCORES)), trace=True)
    return _reduce_results(res), res


# revision 15
# speedup vs baseline: 29.9601x; 29.9601x over previous
"""Trainium2 Bass kernel for ColorImageLoss (gaussian-blur + bilinear grid
sample + MSE), data-parallel over batch across 8 NeuronCores.

Key idea: the loss only reads the blurred image at 64 sample points per
image.  Each bilinear sample needs a 2x2 patch of blurred pixels; the 7-tap
separable blur support of those pixels is an 8x8 patch of the *original*
image (reflect padding folds into per-sample 8-tap row/col weight vectors;
reflected tap indices provably stay inside the clamped 8-wide window
[clamp(x0-3,0,W-8), +8)).

Gather strategy: the host pre-builds a *banded* replica of each image —
for every window row start ys in [0,505) an 8-row band stored as
[x][c][r] — so a full 8x8x3 patch is ONE contiguous 768B run at element
offset ((img*505+ys)*512+xs)*24.  The kernel then needs only SLOTS(=2)
indirect-DMA calls (one 768B descriptor per partition each) instead of
48 calls of 32B descriptors; SWDGE per-call overhead dominated the old
58us kernel.  Weights are applied separably (row-combine then
col-combine) to keep every access pattern <=3 free dims.
"""

import os
import sys

import numpy as np

for _p in ("/opt/trn_rl_repo", "/root/.axon_site/_ro/trn_rl_repo"):
    if os.path.isdir(_p) and _p not in sys.path:
        sys.path.insert(0, _p)

import concourse.bass as bass
import concourse.mybir as mybir
import concourse.tile as tile
from concourse.bass_utils import run_bass_kernel_spmd

# Problem geometry (hardcoded per contract)
B, L, NCH, H, W = 32, 64, 3, 512, 512
NCORES = 8
BPC = B // NCORES            # images per core
NS = BPC * L                 # samples per core (256)
P = 128                      # SBUF partitions
SLOTS = NS // P              # 2 sample slots per partition
KS = 7                       # blur taps
BANDS = H - 8 + 1            # 505 window row starts
BAND_ROW = W * NCH * 8       # 12288 elems per (img, ys)
IMG_BASE = BANDS * BAND_ROW  # 6205440 elems per image
IMG_ELEMS = BPC * IMG_BASE   # banded elems per core
PATCH = 8 * NCH * 8          # 192 elems per gathered patch

f32 = mybir.dt.float32
i32 = mybir.dt.int32
Alu = mybir.AluOpType
Ax = mybir.AxisListType

# meta tensor per-partition layout (f32 columns)
O_POS = 0             # [SLOTS, 2] (x, y)                  -> 4
O_COL = 4             # [SLOTS, 3] color                   -> 6
O_JMW = 10            # [2which, 7] j-3+which              -> 14
O_KK = 24             # [7] blur kernel                    -> 7
O_IOTA8 = 31          # [8] 0..7                           -> 8
O_SCALE = 39          # (24, 12288) x/y index scales       -> 2
O_BASE = 41           # [SLOTS] i32 image base (bit-cast)  -> 2
META_W = 44


def _gauss_kernel_np():
    x = (np.arange(KS, dtype=np.float32) - (KS - 1) / 2).astype(np.float32)
    k = np.exp(-0.5 * (x / np.float32(1.0)) ** 2).astype(np.float32)
    return (k / k.sum()).astype(np.float32)


def _fap(t, dims, extra_offset=0):
    """AP over tile `t` keeping its partition dim, replacing free dims.

    dims: list of [step, count] in elements; step 0 broadcasts.
    """
    base = t[:] if hasattr(t, "tile") else t
    return bass.AP(
        base.tensor, base.offset + extra_offset,
        [list(base.ap[0])] + [list(d) for d in dims],
    )


def split_multi_waits(nc):
    """Walrus encodes at most ONE sync wait per TPB instruction.  Hoist
    extra waits onto same-engine NoOps inserted directly before the
    instruction (the sequencer executes waits in queue order, so semantics
    are identical)."""
    n_split = 0
    for f in nc.m.functions:
        for blk in f.blocks:
            insts = blk.instructions
            i = 0
            while i < len(insts):
                inst = insts[i]
                si = inst.sync_info
                if si is not None and si.on_wait is not None and len(si.on_wait) > 1:
                    waits = list(si.on_wait)
                    for w in waits[:-1]:
                        nop = mybir.InstNoOp(
                            name=f"{inst.name}-wsplit{n_split}",
                            engine=inst.engine,
                            ins=[],
                            outs=[],
                            sync_info=mybir.SyncInfo(on_wait=[w], on_update=[]),
                        )
                        nc.register_instruction(nop, overwrite=True)
                        insts.insert(i, nop)
                        i += 1
                        n_split += 1
                    inst.sync_info = mybir.SyncInfo(
                        on_wait=[waits[-1]], on_update=list(si.on_update or []))
                i += 1
    return n_split


def build_bass(debug_taps=False, repeat=1, skip_gather=False, gather_len=96,
               serialize=True):
    """serialize: chain each repeat body's final result into the next
    body's meta tile so repeats cannot overlap — the repeat-slope then
    measures true per-body latency, not pipelined throughput."""
    assert not (debug_taps and repeat > 1)
    nc = bass.Bass("TRN2")

    img = nc.dram_tensor("img", [IMG_ELEMS, 1], f32, kind="ExternalInput")
    meta = nc.dram_tensor("meta", [P, META_W], f32, kind="ExternalInput")
    partial = nc.dram_tensor("partial", [P, 1], f32, kind="ExternalOutput")

    taps = []

    def tap(name, t, width, dt=f32):
        if not debug_taps:
            return
        d = nc.dram_tensor(f"tap_{name}", [P, width], dt, kind="ExternalOutput")
        taps.append((name, t, d, width))

    with tile.TileContext(nc) as tc:
        with tc.tile_pool(name="main", bufs=1) as pool:
            # tiles allocated ONCE; `repeat` bodies reuse them so reps
            # serialize through buffer dependencies (honest latency bench)
            m = pool.tile([P, META_W], f32)
            xy = pool.tile([P, SLOTS, 2], f32)
            s2 = pool.tile([P, SLOTS, 2], f32)
            offf = pool.tile([P, SLOTS, 2], f32)
            off = pool.tile([P, SLOTS], f32)
            offi = pool.tile([P, SLOTS], i32)
            idx = pool.tile([P, SLOTS], i32)
            patches = pool.tile([P, SLOTS, PATCH], f32)
            wxy = pool.tile([P, SLOTS, 2], f32)
            fxy = pool.tile([P, SLOTS, 2], f32)
            rnd = pool.tile([P, SLOTS, 2], f32)
            gtx = pool.tile([P, SLOTS, 2], f32)
            tt = pool.tile([P, SLOTS, 2, 2 * KS], f32)
            ta = pool.tile([P, SLOTS, 2, 2 * KS], f32)
            tb = pool.tile([P, SLOTS, 2, 2 * KS], f32)
            tr = pool.tile([P, SLOTS, 2, 2 * KS], f32)
            tz = pool.tile([P, SLOTS, 2, 2 * KS], f32)
            eq = pool.tile([P, SLOTS * 4, 8, KS], f32)
            cw = pool.tile([P, SLOTS, 2, 2, KS], f32)
            eqk = pool.tile([P, SLOTS * 4, 8, KS], f32)
            vh = pool.tile([P, SLOTS, 2, 2, 8], f32)
            hwv = pool.tile([P, SLOTS, 2, 8], f32)
            t1 = pool.tile([P, SLOTS, 24, 8], f32)
            t1r = pool.tile([P, SLOTS, 24], f32)
            t2 = pool.tile([P, SLOTS, NCH, 8], f32)
            tgt = pool.tile([P, SLOTS, NCH], f32)
            diff = pool.tile([P, SLOTS, NCH], f32)
            sqj = pool.tile([P, SLOTS, NCH], f32)
            part = pool.tile([P, 1], f32)

            for _rep in range(repeat):
                nc.sync.dma_start(out=m[:], in_=meta[:])

                # ---- index path (gather critical path) ----
                # x = clip(pos*512 - 0.5, 0, 511)  (continuous in pos, so
                # f32 reassociation vs the reference op order is harmless)
                pos_ap = _fap(m, [[2, SLOTS], [1, 2]], O_POS)
                nc.vector.tensor_scalar(xy[:], pos_ap, float(W), -0.5,
                                        Alu.mult, Alu.add)
                nc.vector.tensor_scalar(xy[:], xy[:], 0.0, float(W - 1),
                                        Alu.max, Alu.min)
                tap('xy', xy, 4)
                # floor via round-to-nearest at 2^23 (f32 grid is 1.0 there),
                # minus the rounded-up-past-x case
                nc.vector.tensor_scalar(rnd[:], xy[:], 8388608.0, None, Alu.add)
                nc.vector.tensor_scalar(rnd[:], rnd[:], -8388608.0, None,
                                        Alu.add)
                nc.vector.tensor_tensor(gtx[:], rnd[:], xy[:], op=Alu.is_gt)
                nc.vector.tensor_tensor(fxy[:], rnd[:], gtx[:],
                                        op=Alu.subtract)
                # window start s2 = clamp(floor(x)-3, 0, 504), integral f32
                nc.vector.tensor_scalar(s2[:], fxy[:], -3.0, 0.0,
                                        Alu.add, Alu.max)
                nc.vector.tensor_scalar(s2[:], s2[:], float(W - 8), None,
                                        Alu.min)
                # element offset = sx*24 + sy*12288 (exact in f32: <2^24)
                scale_ap = _fap(m, [[0, SLOTS], [1, 2]], O_SCALE)
                nc.vector.tensor_tensor(offf[:], s2[:], scale_ap, op=Alu.mult)
                nc.vector.tensor_tensor(
                    off[:], _fap(offf, [[2, SLOTS], [1, 1]]),
                    _fap(offf, [[2, SLOTS], [1, 1]], 1), op=Alu.add)
                # i32 trunc of s2 is exact (integral); add per-slot img base
                nc.vector.tensor_copy(offi[:], off[:])
                base_ap = _fap(m, [[1, SLOTS]], O_BASE).bitcast(i32)
                nc.vector.tensor_tensor(idx[:], offi[:], base_ap, op=Alu.add)
                tap('idx', idx, SLOTS, i32)

                # ---- gather: one 768B descriptor per partition per slot ----
                if skip_gather:
                    nc.vector.memset(patches[:], 0.0)
                else:
                    for slot in range(SLOTS):
                        for seg in range(PATCH // gather_len):
                            nc.gpsimd.indirect_dma_start(
                                out=_fap(patches, [[1, gather_len]],
                                         PATCH * slot + gather_len * seg),
                                out_offset=None,
                                in_=img[:],
                                in_offset=bass.IndirectOffsetOnAxis(
                                    ap=_fap(idx, [[1, 1]], slot), axis=0),
                                element_offset=gather_len * seg,
                            )

                # ---- weight path (overlaps gather DMA) ----
                nc.vector.tensor_tensor(wxy[:], xy[:], fxy[:], op=Alu.subtract)
                tap('fxy', fxy, 4)
                # raw taps t[s, ax, w, j] = floor + (j-3+w)
                jmw_ap = _fap(m, [[0, SLOTS], [0, 2], [1, 2 * KS]], O_JMW)
                fxy_b = _fap(fxy, [[2, SLOTS], [1, 2], [0, 2 * KS]])
                nc.vector.tensor_tensor(
                    _fap(tt, [[4 * KS, SLOTS], [2 * KS, 2], [1, 2 * KS]]),
                    fxy_b, jmw_ap, op=Alu.add)
                # reflect: r = min(|t|, 2*(W-1) - t);  |t| = max(t, -t)
                nc.vector.tensor_scalar(ta[:], tt[:], -1.0, None, Alu.mult)
                nc.vector.tensor_tensor(ta[:], ta[:], tt[:], op=Alu.max)
                nc.vector.tensor_scalar(tb[:], tt[:], -1.0, float(2 * (W - 1)),
                                        Alu.mult, Alu.add)
                nc.vector.tensor_tensor(tr[:], ta[:], tb[:], op=Alu.min)
                # window-relative taps z = r - s(ax)  (in [0,8) iff covered)
                s2_b = _fap(s2, [[2, SLOTS], [1, 2], [0, 2 * KS]])
                nc.vector.tensor_tensor(
                    _fap(tz, [[4 * KS, SLOTS], [2 * KS, 2], [1, 2 * KS]]),
                    _fap(tr, [[4 * KS, SLOTS], [2 * KS, 2], [1, 2 * KS]]),
                    s2_b, op=Alu.subtract)
                tap('tz', tz, 4 * KS * SLOTS)
                # eq[g, u, j] = (z[g, j] == u),  g = (s, ax, w)
                G = SLOTS * 4
                z_b = _fap(tz, [[KS, G], [0, 8], [1, KS]])
                iota_b = _fap(m, [[0, G], [1, 8], [0, KS]], O_IOTA8)
                nc.vector.tensor_tensor(eq[:], z_b, iota_b, op=Alu.is_equal)
                # cw[s, ax, w, j] = kk[j] * (w ? wxy : 1-wxy)
                kk_b = _fap(m, [[0, SLOTS], [0, 2], [1, KS]], O_KK)
                wxy_b = _fap(wxy, [[2, SLOTS], [1, 2], [0, KS]])
                cw1 = _fap(cw, [[4 * KS, SLOTS], [2 * KS, 2], [1, KS]], KS)
                nc.vector.tensor_tensor(cw1, kk_b, wxy_b, op=Alu.mult)
                nc.vector.tensor_tensor(
                    _fap(cw, [[4 * KS, SLOTS], [2 * KS, 2], [1, KS]]),
                    kk_b, cw1, op=Alu.subtract)
                # vh[g, u] = sum_j eq * cw ; hwv[s, ax, u] = vh_w0 + vh_w1
                cw_b = _fap(cw, [[KS, G], [0, 8], [1, KS]])
                nc.vector.tensor_tensor(eqk[:], eq[:], cw_b, op=Alu.mult)
                nc.vector.tensor_reduce(
                    out=_fap(vh, [[1, G * 8]]), in_=eqk[:], axis=Ax.X,
                    op=Alu.add)
                nc.vector.tensor_tensor(
                    hwv[:],
                    _fap(vh, [[32, SLOTS], [16, 2], [1, 8]]),
                    _fap(vh, [[32, SLOTS], [16, 2], [1, 8]], 8), op=Alu.add)
                tap('hwv', hwv, SLOTS * 16)

                # ---- apply weights separably; patch layout [x][c][r] ----
                tap('patches', patches, SLOTS * PATCH)
                vw_b = _fap(hwv, [[16, SLOTS], [0, 24], [1, 8]], 8)
                pat_v = _fap(patches, [[PATCH, SLOTS], [8, 24], [1, 8]])
                nc.vector.tensor_tensor(t1[:], pat_v, vw_b, op=Alu.mult)
                nc.vector.tensor_reduce(
                    out=_fap(t1r, [[1, SLOTS * 24]]), in_=t1[:], axis=Ax.X,
                    op=Alu.add)
                hw_b = _fap(hwv, [[16, SLOTS], [0, NCH], [1, 8]])
                t1r_v = _fap(t1r, [[24, SLOTS], [1, NCH], [NCH, 8]])
                nc.vector.tensor_tensor(
                    _fap(t2, [[NCH * 8, SLOTS], [8, NCH], [1, 8]]),
                    t1r_v, hw_b, op=Alu.mult)
                nc.vector.tensor_reduce(
                    out=_fap(tgt, [[1, SLOTS * NCH]]), in_=t2[:], axis=Ax.X,
                    op=Alu.add)
                tap('tgt', tgt, SLOTS * NCH)
                col_ap = _fap(m, [[NCH, SLOTS], [1, NCH]], O_COL)
                nc.vector.tensor_tensor(diff[:], tgt[:], col_ap,
                                        op=Alu.subtract)
                nc.vector.tensor_tensor(sqj[:], diff[:], diff[:], op=Alu.mult)
                nc.vector.tensor_reduce(
                    out=part[:], in_=_fap(sqj, [[1, SLOTS * NCH]]), axis=Ax.X,
                    op=Alu.add)

                nc.sync.dma_start(out=partial[:], in_=part[:])
                if serialize and repeat > 1:
                    # force rep i+1's meta load to wait on rep i's result
                    nc.vector.tensor_copy(_fap(m, [[1, 1]], O_POS), part[:])

                for _name, _t, _d, _w in taps:
                    nc.sync.dma_start(out=_d[:], in_=_fap(_t, [[1, _w]]))

    split_multi_waits(nc)
    return nc


def make_meta(pred_shard):
    """Build the per-core [P, META_W] meta tensor from the [BPC, L, 8]
    predictions shard.  Sample i = slot*P + p."""
    flat = np.ascontiguousarray(pred_shard.reshape(NS, 8).astype(np.float32))
    meta = np.zeros((P, META_W), dtype=np.float32)
    pos = flat[:, :2].reshape(SLOTS, P, 2).transpose(1, 0, 2)     # [P,SLOTS,2]
    col = flat[:, 5:8].reshape(SLOTS, P, 3).transpose(1, 0, 2)    # [P,SLOTS,3]
    meta[:, O_POS:O_POS + 4] = pos.reshape(P, 4)
    meta[:, O_COL:O_COL + 6] = col.reshape(P, 6)
    jmw = np.concatenate([np.arange(KS, dtype=np.float32) - 3.0 + w
                          for w in (0.0, 1.0)])                   # [2*7]
    meta[:, O_JMW:O_JMW + 14] = jmw[None, :]
    meta[:, O_KK:O_KK + KS] = _gauss_kernel_np()[None, :]
    meta[:, O_IOTA8:O_IOTA8 + 8] = np.arange(8, dtype=np.float32)[None, :]
    meta[:, O_SCALE] = float(NCH * 8)        # x scale (24)
    meta[:, O_SCALE + 1] = float(BAND_ROW)   # y scale (12288)
    p_idx = np.arange(P)
    base = np.zeros((P, SLOTS), dtype=np.int32)
    for slot in range(SLOTS):
        base[:, slot] = ((slot * P + p_idx) // L).astype(np.int32) * IMG_BASE
    meta[:, O_BASE:O_BASE + SLOTS] = base.view(np.float32)
    return meta


def make_banded(ref_imgs):
    """banded[i, ys, x, c, r] = img[i, c, ys+r, x]  — an 8x8x3 patch at
    (ys, xs) is the contiguous 192-elem run at ((i*505+ys)*512+xs)*24."""
    imgs = np.asarray(ref_imgs, dtype=np.float32)
    Y = np.ascontiguousarray(imgs.transpose(0, 2, 3, 1))  # [B, y, x, c]
    banded = np.empty((B, BANDS, W, NCH, 8), dtype=np.float32)
    for r in range(8):
        banded[:, :, :, :, r] = Y[:, r:r + BANDS]
    return banded


def make_in_maps(predictions, ref_imgs):
    banded = make_banded(ref_imgs)
    in_maps = []
    for k in range(NCORES):
        shard = np.ascontiguousarray(
            banded[k * BPC:(k + 1) * BPC]).reshape(-1, 1)
        meta = make_meta(predictions[k * BPC:(k + 1) * BPC])
        in_maps.append({"img": shard, "meta": meta})
    return in_maps


_NC_CACHE = {}


def get_nc():
    if "nc" not in _NC_CACHE:
        _NC_CACHE["nc"] = build_bass()
    return _NC_CACHE["nc"]


def _reduce_results(res):
    total = np.float64(0.0)
    for r in res.results:
        total += np.float64(r["partial"].sum(dtype=np.float64))
    return np.float32(total / (B * L * NCH))


def kernel(predictions, ref_imgs):
    predictions = np.asarray(predictions)
    ref_imgs = np.asarray(ref_imgs)
    nc = get_nc()
    in_maps = make_in_maps(predictions, ref_imgs)
    res = run_bass_kernel_spmd(nc, in_maps, list(range(NCORES)))
    return _reduce_results(res)


def run_profiled(predictions, ref_imgs):
    """Like kernel(), but traces with neuron-profile; returns (loss, results)."""
    predictions = np.asarray(predictions)
    ref_imgs = np.asarray(ref_imgs)
    nc = get_nc()
    in_maps = make_in_maps(predictions, ref_imgs)
    res = run_bass_kernel_spmd(
        nc, in_maps, list(range(NCORES)), trace=True)
    return _reduce_results(res), res


# revision 17
# speedup vs baseline: 169.2892x; 5.6505x over previous
"""Trainium2 Bass kernel for ColorImageLoss (gaussian-blur + bilinear grid
sample + MSE), data-parallel over batch across 8 NeuronCores.

Key idea: the loss only reads the blurred image at 64 sample points per
image.  Each bilinear sample needs a 2x2 patch of blurred pixels; the 7-tap
separable blur support of those pixels is an 8x8 patch of the *original*
image (reflect padding folds into per-sample 8-tap row/col weight vectors;
reflected tap indices provably stay inside the clamped 8-wide window
[clamp(x0-3,0,W-8), +8)).

Gather strategy: the host pre-builds a *banded* replica of each image —
for every window row start ys in [0,505) an 8-row band stored as
[x][c][r] — so a full 8x8x3 patch is ONE contiguous 768B run at element
offset ((img*505+ys)*512+xs)*24.  The kernel then needs only SLOTS(=2)
indirect-DMA calls (one 768B descriptor per partition each) instead of
48 calls of 32B descriptors; SWDGE per-call overhead dominated the old
58us kernel.  Weights are applied separably (row-combine then
col-combine) to keep every access pattern <=3 free dims.
"""

import os
import sys

import numpy as np

for _p in ("/opt/trn_rl_repo", "/root/.axon_site/_ro/trn_rl_repo"):
    if os.path.isdir(_p) and _p not in sys.path:
        sys.path.insert(0, _p)

import concourse.bass as bass
import concourse.mybir as mybir
import concourse.tile as tile
from concourse.bass_utils import run_bass_kernel_spmd

# Problem geometry (hardcoded per contract)
B, L, NCH, H, W = 32, 64, 3, 512, 512
NCORES = 8
BPC = B // NCORES            # images per core
NS = BPC * L                 # samples per core (256)
P = 128                      # SBUF partitions
SLOTS = NS // P              # 2 sample slots per partition
KS = 7                       # blur taps
BANDS = H - 8 + 1            # 505 window row starts
BAND_ROW = W * NCH * 8       # 12288 elems per (img, ys)
IMG_BASE = BANDS * BAND_ROW  # 6205440 elems per image
IMG_ELEMS = BPC * IMG_BASE   # banded elems per core
PATCH = 8 * NCH * 8          # 192 elems per gathered patch

f32 = mybir.dt.float32
i32 = mybir.dt.int32
Alu = mybir.AluOpType
Ax = mybir.AxisListType

# meta tensor per-partition layout (f32 columns)
O_POS = 0             # [SLOTS, 2] (x, y)                  -> 4
O_COL = 4             # [SLOTS, 3] color                   -> 6
O_JMW = 10            # [2which, 7] j-3+which              -> 14
O_KK = 24             # [7] blur kernel                    -> 7
O_IOTA8 = 31          # [8] 0..7                           -> 8
O_SCALE = 39          # (24, 12288) x/y index scales       -> 2
O_BASE = 41           # [SLOTS] i32 image base (bit-cast)  -> 2
META_W = 44


def _gauss_kernel_np():
    x = (np.arange(KS, dtype=np.float32) - (KS - 1) / 2).astype(np.float32)
    k = np.exp(-0.5 * (x / np.float32(1.0)) ** 2).astype(np.float32)
    return (k / k.sum()).astype(np.float32)


def _fap(t, dims, extra_offset=0):
    """AP over tile `t` keeping its partition dim, replacing free dims.

    dims: list of [step, count] in elements; step 0 broadcasts.
    """
    base = t[:] if hasattr(t, "tile") else t
    return bass.AP(
        base.tensor, base.offset + extra_offset,
        [list(base.ap[0])] + [list(d) for d in dims],
    )


def split_multi_waits(nc):
    """Walrus encodes at most ONE sync wait per TPB instruction.  Hoist
    extra waits onto same-engine NoOps inserted directly before the
    instruction (the sequencer executes waits in queue order, so semantics
    are identical)."""
    n_split = 0
    for f in nc.m.functions:
        for blk in f.blocks:
            insts = blk.instructions
            i = 0
            while i < len(insts):
                inst = insts[i]
                si = inst.sync_info
                if si is not None and si.on_wait is not None and len(si.on_wait) > 1:
                    waits = list(si.on_wait)
                    for w in waits[:-1]:
                        nop = mybir.InstNoOp(
                            name=f"{inst.name}-wsplit{n_split}",
                            engine=inst.engine,
                            ins=[],
                            outs=[],
                            sync_info=mybir.SyncInfo(on_wait=[w], on_update=[]),
                        )
                        nc.register_instruction(nop, overwrite=True)
                        insts.insert(i, nop)
                        i += 1
                        n_split += 1
                    inst.sync_info = mybir.SyncInfo(
                        on_wait=[waits[-1]], on_update=list(si.on_update or []))
                i += 1
    return n_split


def build_bass(debug_taps=False, repeat=1, skip_gather=False, gather_len=96,
               serialize=True, img_elems=IMG_ELEMS):
    """serialize: chain each repeat body's final result into the next
    body's meta tile so repeats cannot overlap — the repeat-slope then
    measures true per-body latency, not pipelined throughput."""
    assert not (debug_taps and repeat > 1)
    nc = bass.Bass("TRN2")

    img = nc.dram_tensor("img", [img_elems, 1], f32, kind="ExternalInput")
    meta = nc.dram_tensor("meta", [P, META_W], f32, kind="ExternalInput")
    partial = nc.dram_tensor("partial", [P, 1], f32, kind="ExternalOutput")

    taps = []

    def tap(name, t, width, dt=f32):
        if not debug_taps:
            return
        d = nc.dram_tensor(f"tap_{name}", [P, width], dt, kind="ExternalOutput")
        taps.append((name, t, d, width))

    with tile.TileContext(nc) as tc:
        with tc.tile_pool(name="main", bufs=1) as pool:
            # tiles allocated ONCE; `repeat` bodies reuse them so reps
            # serialize through buffer dependencies (honest latency bench)
            m = pool.tile([P, META_W], f32)
            xy = pool.tile([P, SLOTS, 2], f32)
            s2 = pool.tile([P, SLOTS, 2], f32)
            offf = pool.tile([P, SLOTS, 2], f32)
            off = pool.tile([P, SLOTS], f32)
            offi = pool.tile([P, SLOTS], i32)
            idx = pool.tile([P, SLOTS], i32)
            patches = pool.tile([P, SLOTS, PATCH], f32)
            wxy = pool.tile([P, SLOTS, 2], f32)
            fxy = pool.tile([P, SLOTS, 2], f32)
            rnd = pool.tile([P, SLOTS, 2], f32)
            gtx = pool.tile([P, SLOTS, 2], f32)
            tt = pool.tile([P, SLOTS, 2, 2 * KS], f32)
            ta = pool.tile([P, SLOTS, 2, 2 * KS], f32)
            tb = pool.tile([P, SLOTS, 2, 2 * KS], f32)
            tr = pool.tile([P, SLOTS, 2, 2 * KS], f32)
            tz = pool.tile([P, SLOTS, 2, 2 * KS], f32)
            eq = pool.tile([P, SLOTS * 4, 8, KS], f32)
            cw = pool.tile([P, SLOTS, 2, 2, KS], f32)
            eqk = pool.tile([P, SLOTS * 4, 8, KS], f32)
            vh = pool.tile([P, SLOTS, 2, 2, 8], f32)
            hwv = pool.tile([P, SLOTS, 2, 8], f32)
            t1 = pool.tile([P, SLOTS, 24, 8], f32)
            t1r = pool.tile([P, SLOTS, 24], f32)
            t2 = pool.tile([P, SLOTS, NCH, 8], f32)
            tgt = pool.tile([P, SLOTS, NCH], f32)
            diff = pool.tile([P, SLOTS, NCH], f32)
            sqj = pool.tile([P, SLOTS, NCH], f32)
            part = pool.tile([P, 1], f32)

            for _rep in range(repeat):
                nc.sync.dma_start(out=m[:], in_=meta[:])

                # ---- index path (gather critical path) ----
                # x = clip(pos*512 - 0.5, 0, 511)  (continuous in pos, so
                # f32 reassociation vs the reference op order is harmless)
                pos_ap = _fap(m, [[2, SLOTS], [1, 2]], O_POS)
                nc.vector.tensor_scalar(xy[:], pos_ap, float(W), -0.5,
                                        Alu.mult, Alu.add)
                nc.vector.tensor_scalar(xy[:], xy[:], 0.0, float(W - 1),
                                        Alu.max, Alu.min)
                tap('xy', xy, 4)
                # floor via round-to-nearest at 2^23 (f32 grid is 1.0 there),
                # minus the rounded-up-past-x case
                nc.vector.tensor_scalar(rnd[:], xy[:], 8388608.0, None, Alu.add)
                nc.vector.tensor_scalar(rnd[:], rnd[:], -8388608.0, None,
                                        Alu.add)
                nc.vector.tensor_tensor(gtx[:], rnd[:], xy[:], op=Alu.is_gt)
                nc.vector.tensor_tensor(fxy[:], rnd[:], gtx[:],
                                        op=Alu.subtract)
                # window start s2 = clamp(floor(x)-3, 0, 504), integral f32
                nc.vector.tensor_scalar(s2[:], fxy[:], -3.0, 0.0,
                                        Alu.add, Alu.max)
                nc.vector.tensor_scalar(s2[:], s2[:], float(W - 8), None,
                                        Alu.min)
                # element offset = sx*24 + sy*12288 (exact in f32: <2^24)
                scale_ap = _fap(m, [[0, SLOTS], [1, 2]], O_SCALE)
                nc.vector.tensor_tensor(offf[:], s2[:], scale_ap, op=Alu.mult)
                nc.vector.tensor_tensor(
                    off[:], _fap(offf, [[2, SLOTS], [1, 1]]),
                    _fap(offf, [[2, SLOTS], [1, 1]], 1), op=Alu.add)
                # i32 trunc of s2 is exact (integral); add per-slot img base
                nc.vector.tensor_copy(offi[:], off[:])
                base_ap = _fap(m, [[1, SLOTS]], O_BASE).bitcast(i32)
                nc.vector.tensor_tensor(idx[:], offi[:], base_ap, op=Alu.add)
                tap('idx', idx, SLOTS, i32)

                # ---- gather: one 768B descriptor per partition per slot ----
                if skip_gather:
                    nc.vector.memset(patches[:], 0.0)
                else:
                    for slot in range(SLOTS):
                        for seg in range(PATCH // gather_len):
                            nc.gpsimd.indirect_dma_start(
                                out=_fap(patches, [[1, gather_len]],
                                         PATCH * slot + gather_len * seg),
                                out_offset=None,
                                in_=img[:],
                                in_offset=bass.IndirectOffsetOnAxis(
                                    ap=_fap(idx, [[1, 1]], slot), axis=0),
                                element_offset=gather_len * seg,
                            )

                # ---- weight path (overlaps gather DMA) ----
                nc.vector.tensor_tensor(wxy[:], xy[:], fxy[:], op=Alu.subtract)
                tap('fxy', fxy, 4)
                # raw taps t[s, ax, w, j] = floor + (j-3+w)
                jmw_ap = _fap(m, [[0, SLOTS], [0, 2], [1, 2 * KS]], O_JMW)
                fxy_b = _fap(fxy, [[2, SLOTS], [1, 2], [0, 2 * KS]])
                nc.vector.tensor_tensor(
                    _fap(tt, [[4 * KS, SLOTS], [2 * KS, 2], [1, 2 * KS]]),
                    fxy_b, jmw_ap, op=Alu.add)
                # reflect: r = min(|t|, 2*(W-1) - t);  |t| = max(t, -t)
                nc.vector.tensor_scalar(ta[:], tt[:], -1.0, None, Alu.mult)
                nc.vector.tensor_tensor(ta[:], ta[:], tt[:], op=Alu.max)
                nc.vector.tensor_scalar(tb[:], tt[:], -1.0, float(2 * (W - 1)),
                                        Alu.mult, Alu.add)
                nc.vector.tensor_tensor(tr[:], ta[:], tb[:], op=Alu.min)
                # window-relative taps z = r - s(ax)  (in [0,8) iff covered)
                s2_b = _fap(s2, [[2, SLOTS], [1, 2], [0, 2 * KS]])
                nc.vector.tensor_tensor(
                    _fap(tz, [[4 * KS, SLOTS], [2 * KS, 2], [1, 2 * KS]]),
                    _fap(tr, [[4 * KS, SLOTS], [2 * KS, 2], [1, 2 * KS]]),
                    s2_b, op=Alu.subtract)
                tap('tz', tz, 4 * KS * SLOTS)
                # eq[g, u, j] = (z[g, j] == u),  g = (s, ax, w)
                G = SLOTS * 4
                z_b = _fap(tz, [[KS, G], [0, 8], [1, KS]])
                iota_b = _fap(m, [[0, G], [1, 8], [0, KS]], O_IOTA8)
                nc.vector.tensor_tensor(eq[:], z_b, iota_b, op=Alu.is_equal)
                # cw[s, ax, w, j] = kk[j] * (w ? wxy : 1-wxy)
                kk_b = _fap(m, [[0, SLOTS], [0, 2], [1, KS]], O_KK)
                wxy_b = _fap(wxy, [[2, SLOTS], [1, 2], [0, KS]])
                cw1 = _fap(cw, [[4 * KS, SLOTS], [2 * KS, 2], [1, KS]], KS)
                nc.vector.tensor_tensor(cw1, kk_b, wxy_b, op=Alu.mult)
                nc.vector.tensor_tensor(
                    _fap(cw, [[4 * KS, SLOTS], [2 * KS, 2], [1, KS]]),
                    kk_b, cw1, op=Alu.subtract)
                # vh[g, u] = sum_j eq * cw ; hwv[s, ax, u] = vh_w0 + vh_w1
                cw_b = _fap(cw, [[KS, G], [0, 8], [1, KS]])
                nc.vector.tensor_tensor(eqk[:], eq[:], cw_b, op=Alu.mult)
                nc.vector.tensor_reduce(
                    out=_fap(vh, [[1, G * 8]]), in_=eqk[:], axis=Ax.X,
                    op=Alu.add)
                nc.vector.tensor_tensor(
                    hwv[:],
                    _fap(vh, [[32, SLOTS], [16, 2], [1, 8]]),
                    _fap(vh, [[32, SLOTS], [16, 2], [1, 8]], 8), op=Alu.add)
                tap('hwv', hwv, SLOTS * 16)

                # ---- apply weights separably; patch layout [x][c][r] ----
                tap('patches', patches, SLOTS * PATCH)
                vw_b = _fap(hwv, [[16, SLOTS], [0, 24], [1, 8]], 8)
                pat_v = _fap(patches, [[PATCH, SLOTS], [8, 24], [1, 8]])
                nc.vector.tensor_tensor(t1[:], pat_v, vw_b, op=Alu.mult)
                nc.vector.tensor_reduce(
                    out=_fap(t1r, [[1, SLOTS * 24]]), in_=t1[:], axis=Ax.X,
                    op=Alu.add)
                hw_b = _fap(hwv, [[16, SLOTS], [0, NCH], [1, 8]])
                t1r_v = _fap(t1r, [[24, SLOTS], [1, NCH], [NCH, 8]])
                nc.vector.tensor_tensor(
                    _fap(t2, [[NCH * 8, SLOTS], [8, NCH], [1, 8]]),
                    t1r_v, hw_b, op=Alu.mult)
                nc.vector.tensor_reduce(
                    out=_fap(tgt, [[1, SLOTS * NCH]]), in_=t2[:], axis=Ax.X,
                    op=Alu.add)
                tap('tgt', tgt, SLOTS * NCH)
                col_ap = _fap(m, [[NCH, SLOTS], [1, NCH]], O_COL)
                nc.vector.tensor_tensor(diff[:], tgt[:], col_ap,
                                        op=Alu.subtract)
                nc.vector.tensor_tensor(sqj[:], diff[:], diff[:], op=Alu.mult)
                nc.vector.tensor_reduce(
                    out=part[:], in_=_fap(sqj, [[1, SLOTS * NCH]]), axis=Ax.X,
                    op=Alu.add)

                nc.sync.dma_start(out=partial[:], in_=part[:])
                if serialize and repeat > 1:
                    # force rep i+1's meta load to wait on rep i's result
                    nc.vector.tensor_copy(_fap(m, [[1, 1]], O_POS), part[:])

                for _name, _t, _d, _w in taps:
                    nc.sync.dma_start(out=_d[:], in_=_fap(_t, [[1, _w]]))

    split_multi_waits(nc)
    return nc


def make_meta(pred_shard):
    """Build the per-core [P, META_W] meta tensor from the [BPC, L, 8]
    predictions shard.  Sample i = slot*P + p."""
    flat = np.ascontiguousarray(pred_shard.reshape(NS, 8).astype(np.float32))
    meta = np.zeros((P, META_W), dtype=np.float32)
    pos = flat[:, :2].reshape(SLOTS, P, 2).transpose(1, 0, 2)     # [P,SLOTS,2]
    col = flat[:, 5:8].reshape(SLOTS, P, 3).transpose(1, 0, 2)    # [P,SLOTS,3]
    meta[:, O_POS:O_POS + 4] = pos.reshape(P, 4)
    meta[:, O_COL:O_COL + 6] = col.reshape(P, 6)
    jmw = np.concatenate([np.arange(KS, dtype=np.float32) - 3.0 + w
                          for w in (0.0, 1.0)])                   # [2*7]
    meta[:, O_JMW:O_JMW + 14] = jmw[None, :]
    meta[:, O_KK:O_KK + KS] = _gauss_kernel_np()[None, :]
    meta[:, O_IOTA8:O_IOTA8 + 8] = np.arange(8, dtype=np.float32)[None, :]
    meta[:, O_SCALE] = float(NCH * 8)        # x scale (24)
    meta[:, O_SCALE + 1] = float(BAND_ROW)   # y scale (12288)
    p_idx = np.arange(P)
    base = np.zeros((P, SLOTS), dtype=np.int32)
    for slot in range(SLOTS):
        base[:, slot] = ((slot * P + p_idx) // L).astype(np.int32) * IMG_BASE
    meta[:, O_BASE:O_BASE + SLOTS] = base.view(np.float32)
    return meta


def make_banded(ref_imgs):
    """banded[i, ys, x, c, r] = img[i, c, ys+r, x]  — an 8x8x3 patch at
    (ys, xs) is the contiguous 192-elem run at ((i*505+ys)*512+xs)*24."""
    imgs = np.asarray(ref_imgs, dtype=np.float32)
    Y = np.ascontiguousarray(imgs.transpose(0, 2, 3, 1))  # [B, y, x, c]
    banded = np.empty((B, BANDS, W, NCH, 8), dtype=np.float32)
    for r in range(8):
        banded[:, :, :, :, r] = Y[:, r:r + BANDS]
    return banded


def make_in_maps(predictions, ref_imgs):
    banded = make_banded(ref_imgs)
    in_maps = []
    for k in range(NCORES):
        shard = np.ascontiguousarray(
            banded[k * BPC:(k + 1) * BPC]).reshape(-1, 1)
        meta = make_meta(predictions[k * BPC:(k + 1) * BPC])
        in_maps.append({"img": shard, "meta": meta})
    return in_maps


_NC_CACHE = {}


def get_nc():
    if "nc" not in _NC_CACHE:
        _NC_CACHE["nc"] = build_bass()
    return _NC_CACHE["nc"]


def _reduce_results(res):
    total = np.float64(0.0)
    for r in res.results:
        total += np.float64(r["partial"].sum(dtype=np.float64))
    return np.float32(total / (B * L * NCH))


def kernel(predictions, ref_imgs):
    predictions = np.asarray(predictions)
    ref_imgs = np.asarray(ref_imgs)
    nc = get_nc()
    in_maps = make_in_maps(predictions, ref_imgs)
    res = run_bass_kernel_spmd(nc, in_maps, list(range(NCORES)))
    return _reduce_results(res)


def run_profiled(predictions, ref_imgs):
    """Like kernel(), but traces with neuron-profile; returns (loss, results)."""
    predictions = np.asarray(predictions)
    ref_imgs = np.asarray(ref_imgs)
    nc = get_nc()
    in_maps = make_in_maps(predictions, ref_imgs)
    res = run_bass_kernel_spmd(
        nc, in_maps, list(range(NCORES)), trace=True)
    return _reduce_results(res), res
